# revision 4
# baseline (speedup 1.0000x reference)
"""Trainium2 Bass kernel for nn_Conv2dLocal (locally-connected 2d conv,
no weight sharing).

Strategy: shard the 32 output rows across 8 NeuronCores (4 rows each).
Within a core, the 4 output rows form two PE row-groups (oh{0,1} on array
rows 0-63 with an x copy on SBUF partitions 0-63, oh{2,3} on rows 64-127)
that run concurrently; inside a group the two oh rows map to the two PE
column-halves (also concurrent). Each x pixel [c=64, b=64] is loaded as
the stationary operand once and reused by up to 6 matmuls; the per-pixel
weight slices stream as the moving operand (weights are use-once, so they
ride the fast matmul path instead of LDWEIGHTS). Weights are host-packed
pixel-major so every matmul's rhs is a contiguous fp16 slice and DMA
arrives in 2x 4.7MB chunks. Bias is folded in with a K=2 matmul that
opens each PSUM bank (start=True covers all 128 partitions - the
has_written clear is partition-scoped); drains are 8 [128,512] fp32->fp16
DVE copies each followed by a per-block output DMA.
"""

import numpy as np

import concourse.mybir as mybir
import concourse.tile as tile
from concourse import bacc
from concourse.bass_utils import run_bass_kernel_spmd

B = 64
C = 64
O = 64
OW = 32
N_CORES = 8
R = 4          # output rows per core
XW = 34        # padded pixel columns (w' = -1..32)
XCOLS = 6 * XW * B           # 13056
WCOLS = 32 * 2 * 3 * 3 * 64  # 36864
WCHUNK = WCOLS // 4          # 9216 (8 pixels; nchunks=2 doubles this)
F16 = mybir.dt.float16
F32 = mybir.dt.float32

_NC_CACHE = {}


def _mm_descs():
    """Per-pixel matmul descriptors: (wp, g, hl, ohlp, s0, s1, blk)."""
    out = []
    for wp in range(32):
        i_lo = 1 if wp == 0 else 0
        i_hi = 1 if wp == 31 else 2
        segs = []
        s = i_lo
        for i in range(i_lo, i_hi + 1):
            if (wp - 1 + i) // 8 != (wp - 1 + s) // 8:
                segs.append((s, i - 1))
                s = i
        segs.append((s, i_hi))
        for g in (0, 1):
            rows = range(0, 4) if g == 0 else range(2, 6)
            for hl in rows:
                for ohlp in (0, 1):
                    if not 0 <= hl - 2 * g - ohlp <= 2:
                        continue
                    for (s0, s1) in segs:
                        out.append((wp, g, hl, ohlp, s0, s1, (wp - 1 + s0) // 8))
    return out


def build(n_iter=1, w_bufs=2, ps_bufs=8, nchunks=2):
    nc = bacc.Bacc("TRN2", target_bir_lowering=False, debug=False,
                   num_devices=N_CORES)
    x_d = nc.dram_tensor("xp", [64, XCOLS], F16, kind="ExternalInput")
    w_d = nc.dram_tensor("wt", [128, WCOLS], F16, kind="ExternalInput")
    b_d = nc.dram_tensor("bias", [4, 2048], F16, kind="ExternalInput")
    e_d = nc.dram_tensor("ones", [4, 128], F16, kind="ExternalInput")
    o_d = nc.dram_tensor("out", [2, 128, 2048], F16, kind="ExternalOutput")

    with tile.TileContext(nc) as tc:
        with (
            tc.tile_pool(name="xpool", bufs=1) as xpool,
            tc.tile_pool(name="cpool", bufs=1) as cpool,
            tc.tile_pool(name="opool", bufs=1) as opool,
            tc.tile_pool(name="wpool", bufs=w_bufs) as wpool,
            tc.tile_pool(name="pspool", bufs=ps_bufs, space="PSUM") as pspool,
        ):
            x_sb = xpool.tile([128, XCOLS], F16)
            nc.sync.dma_start(out=x_sb[0:64, :], in_=x_d[:, :])
            nc.sync.dma_start(out=x_sb[64:128, :], in_=x_d[:, :])
            bias_sb = cpool.tile([128, 2048], F16, tag="bias")
            ones_sb = cpool.tile([128, 128], F16, tag="ones")
            for g in (0, 1):
                nc.sync.dma_start(out=bias_sb[64 * g : 64 * g + 2, :],
                                  in_=b_d[2 * g : 2 * g + 2, :])
                nc.sync.dma_start(out=ones_sb[64 * g : 64 * g + 2, :],
                                  in_=e_d[2 * g : 2 * g + 2, :])
            out_sb = [
                opool.tile([128, 2048], F16, tag=f"out{g}", name=f"out_sb{g}")
                for g in (0, 1)
            ]

            descs = _mm_descs()
            n_per_bank = {}
            for d in descs:
                key = (d[1], d[6])
                n_per_bank[key] = n_per_bank.get(key, 0) + 1
            by_pixel = {}
            for d in descs:
                by_pixel.setdefault(d[0], []).append(d)

            wchunk = WCOLS // nchunks
            pix_per_chunk = 32 // nchunks

            def body():
                chunk = [None] * nchunks
                pt = {}
                seen = {}

                for wp in range(32):
                    wl = wp + 1
                    cb = wp // pix_per_chunk
                    if chunk[cb] is None:
                        t = wpool.tile([128, wchunk], F16, name="wt_t")
                        nc.sync.dma_start(
                            out=t[:], in_=w_d[:, cb * wchunk : (cb + 1) * wchunk])
                        chunk[cb] = t
                    for (_, g, hl, ohlp, s0, s1, blk) in by_pixel[wp]:
                        p0 = 64 * g
                        if (g, blk) not in pt:
                            # bias matmul opens the bank: start=True writes
                            # every element (has_written clear is
                            # partition-scoped); x matmuls then accumulate.
                            t = pspool.tile([128, 512], F32, name="ps")
                            pt[(g, blk)] = t
                            seen[(g, blk)] = 0
                            nc.tensor.matmul(
                                t[0:128, 0:512],
                                ones_sb[64 * g : 64 * g + 2, 0:128],
                                bias_sb[64 * g : 64 * g + 2,
                                        blk * 512 : blk * 512 + 512],
                                start=True, stop=False)
                        t = pt[(g, blk)]
                        seen[(g, blk)] += 1
                        last = seen[(g, blk)] == n_per_bank[(g, blk)]
                        kh = hl - 2 * g - ohlp
                        n = (s1 - s0 + 1) * 64
                        fo = ((wp - 1 + s0) % 8) * 64
                        base = (((wp * 2 + ohlp) * 3 + kh) * 3) * 64 - cb * wchunk
                        nc.tensor.matmul(
                            t[64 * ohlp : 64 * ohlp + 64, fo : fo + n],
                            x_sb[p0 : p0 + 64,
                                 (hl * XW + wl) * B : (hl * XW + wl) * B + B],
                            chunk[cb][p0 : p0 + 64,
                                      base + s0 * 64 : base + s0 * 64 + n],
                            start=False, stop=last)
                    if wp >= 8 and (wp - 8) % 8 == 0:
                        for g in (0, 1):
                            blk = wp // 8 - 1
                            nc.vector.tensor_copy(
                                out=out_sb[g][:, blk * 512 : blk * 512 + 512],
                                in_=pt[(g, blk)][:, :])
                            nc.sync.dma_start(
                                out=o_d[g][:, blk * 512 : blk * 512 + 512],
                                in_=out_sb[g][:, blk * 512 : blk * 512 + 512])
                for g in (0, 1):
                    nc.vector.tensor_copy(
                        out=out_sb[g][:, 3 * 512 : 4 * 512], in_=pt[(g, 3)][:, :])
                    nc.sync.dma_start(
                        out=o_d[g][:, 3 * 512 : 4 * 512],
                        in_=out_sb[g][:, 3 * 512 : 4 * 512])

            if n_iter == 1:
                body()
            else:
                with tc.For_i(0, n_iter, 1,
                              hint_engines=(mybir.EngineType.PE,)):
                    body()
    nc.compile()
    return nc


def get_nc():
    if "nc" not in _NC_CACHE:
        _NC_CACHE["nc"] = build()
    return _NC_CACHE["nc"]


# ---------------- host-side layout prep ----------------

def prep_x(x):
    xt = x.transpose(1, 2, 3, 0)  # [c, h, w, b]
    xp = np.zeros((C, 34, 34, B), np.float16)
    xp[:, 1:33, 1:33, :] = xt
    return [
        np.ascontiguousarray(xp[:, R * c : R * c + 6, :, :].reshape(C, XCOLS))
        for c in range(N_CORES)
    ]


def prep_w(weight):
    outs = []
    for core in range(N_CORES):
        r0 = R * core
        Wc = weight[r0 : r0 + 4]                       # [4, 32, O, C, KH, KW]
        T = Wc.transpose(0, 1, 4, 5, 3, 2)             # [ohl, ow, kh, kw, c, o]
        halves = []
        for g in (0, 1):
            wt_g = np.zeros((32, 2, 3, 3, C, O), np.float32)
            for i in (0, 1, 2):
                kw = 2 - i
                lo, hi = max(0, 1 - i), min(32, 33 - i)
                wt_g[lo:hi, :, :, i] = T[2 * g : 2 * g + 2,
                                         lo - 1 + i : hi - 1 + i, :, kw
                                         ].transpose(1, 0, 2, 3, 4)
            halves.append(
                wt_g.reshape(32 * 2 * 3 * 3, C, O).transpose(1, 0, 2)
                .reshape(C, WCOLS))
        outs.append(np.ascontiguousarray(
            np.concatenate(halves, axis=0)).astype(np.float16))
    return outs


def prep_bias(bias):
    outs = []
    for core in range(N_CORES):
        bc = bias[:, R * core : R * core + 4, :]       # [O, 4, OW]
        rows = [np.ascontiguousarray(bc[:, r, :].T).reshape(2048)
                for r in range(4)]                     # [ow, o] flattened
        outs.append(np.stack(rows).astype(np.float16))
    return outs


def prep_ones():
    e = np.zeros((4, 128), np.float16)
    e[0, 0:64] = 1.0
    e[1, 64:128] = 1.0
    e[2, 0:64] = 1.0
    e[3, 64:128] = 1.0
    return e


def make_in_maps(x, weight, bias):
    xs = prep_x(np.asarray(x, np.float32))
    ws = prep_w(np.asarray(weight, np.float32))
    bs = prep_bias(np.asarray(bias, np.float32))
    e = prep_ones()
    return [{"xp": xs[c], "wt": ws[c], "bias": bs[c], "ones": e}
            for c in range(N_CORES)]


def assemble_out(per_core):
    out = np.empty((B, O, 32, OW), np.float32)
    for core in range(N_CORES):
        r0 = R * core
        dev = np.asarray(per_core[core], np.float32).reshape(2, 2, B, OW, O)
        for g in (0, 1):
            for ohlp in (0, 1):
                out[:, :, r0 + 2 * g + ohlp, :] = dev[g, ohlp].transpose(0, 2, 1)
    return out


def kernel(x, weight, bias):
    nc = get_nc()
    in_maps = make_in_maps(x, weight, bias)
    res = run_bass_kernel_spmd(nc, in_maps, core_ids=list(range(N_CORES)))
    return assemble_out([res.results[c]["out"] for c in range(N_CORES)])
